# revision 17
# baseline (speedup 1.0000x reference)
"""DenseCRF mean-field on 8 Trainium2 NeuronCores.

Strategy (1D row-parallel, per the sharding hint):
- The dense N x N kernel K = w_s*Gs + w_b*Gb (diagonal INCLUDED, value 13)
  is built once on-device in bf16 as per-core KT blocks [k, r] (K is
  symmetric, so the [all-k, core-rows] block equals the core's K rows
  transposed). Part of each block stays resident in SBUF; the rest
  streams from core-local DRAM each iteration.
- Each mean-field iteration: M^T[c, r] = sum_k Q[k, c] * K[k, r] on PE
  (Q tile stationary, K streaming), PE-transpose to [r, c], softmax over
  classes with the diagonal removed via A = M - (U + 13*Q_prev_bf16),
  then an 8-core AllGather of the new Q shard ([1568, 21] bf16).
- exp arguments are built with split-bf16 features (3-way hi/mid/lo) so
  the PE cross-term matmul is exact to ~1e-5 despite bf16 operands:
  phi_k . psi_r = f_k . f_r - 0.5||f_r||^2, ACT adds per-partition bias
  -0.5||f_k||^2 + ln(w) and applies exp.
- Position-derived data (spatial phi/bias, the x/y rows of the bilateral
  phi) is baked into the NEFF as inline constants; duplicated feature
  rows are deduplicated on the wire and re-assembled on device, so the
  per-call upload is only the image/unary-derived bytes.
"""

import math

import numpy as np
import ml_dtypes

BF = ml_dtypes.bfloat16

H_IMG, W_IMG, C = 112, 112, 21
N = H_IMG * W_IMG            # 12544
NCORES = 8
R = N // NCORES              # 1568 rows per core
CHUNKS = N // 128            # 98 contraction chunks
RT_FULL = R // 128           # 12 full row tiles
RT_LAST = R - RT_FULL * 128  # 32
RTILES = RT_FULL + 1         # 13
SBUF_CHUNKS = 46             # K chunks kept resident in SBUF per core
BLOCKS = [(0, 512), (512, 512), (1024, 512), (1536, 32)]

PS_ROWS = 9 * 2 + 3          # spatial phi/psi rows (D=2, 3-way split)
PB_ROWS = 9 * 5 + 3          # bilateral phi/psi rows (D=5)

THETA_ALPHA = 80.0
THETA_BETA = 13.0
THETA_GAMMA = 3.0
W_BILATERAL = 10.0
W_SPATIAL = 3.0

# inputs identical across cores (uploaded once, replicated)
_REPLICATED = {"rgbsplit", "biasB"}

_runner_cache = {}


def _split3(v):
    hi = v.astype(BF).astype(np.float64)
    mid = (v - hi).astype(BF).astype(np.float64)
    lo = (v - hi - mid).astype(BF).astype(np.float64)
    return hi, mid, lo


def _pos_features():
    yy, xx = np.meshgrid(np.arange(H_IMG, dtype=np.float64),
                         np.arange(W_IMG, dtype=np.float64), indexing="ij")
    pos = np.stack([xx, yy], axis=-1).reshape(N, 2)
    ctr = (np.array([W_IMG, H_IMG], np.float64) - 1.0) / 2.0
    return pos - ctr


def _spatial_consts():
    """Input-independent spatial data: phi [21, N] bf16, psi-unique [9, N]
    bf16 (a2, b2, c2, L3), bias [CHUNKS, 128] f32."""
    fs = _pos_features() / THETA_GAMMA            # [N, 2]
    a, b, c = _split3(fs.T)
    L = -0.5 * (fs * fs).sum(1)
    Lh, Lm, Ll = _split3(L)
    ones = np.ones((1, N))
    phi = np.concatenate([a, a, a, b, b, b, c, c, c, ones, ones, ones], 0)
    psi_u = np.concatenate([a, b, c, Lh[None], Lm[None], Ll[None]], 0)
    bias = (L + math.log(W_SPATIAL)).astype(np.float32).reshape(CHUNKS, 128)
    return phi.astype(BF), psi_u.astype(BF), bias


def _xy80_const():
    """x/y rows of the bilateral phi + ones rows: [9, N] bf16 =
    (x, y) x (h, m, l) then three ones rows."""
    fxy = _pos_features() / THETA_ALPHA           # [N, 2]
    h, m, l = _split3(fxy.T)                      # [2, N] each
    ones = np.ones((3, N))
    return np.concatenate([h, m, l, ones], 0).astype(BF)


def _build_program(iters):
    import concourse.bacc as bacc
    import concourse.tile as tile
    from concourse import mybir
    from concourse.masks import make_identity

    f32 = mybir.dt.float32
    bf16 = mybir.dt.bfloat16
    AX = mybir.AxisListType.X
    OP = mybir.AluOpType
    EXP = mybir.ActivationFunctionType.Exp

    nc = bacc.Bacc("TRN2", target_bir_lowering=False, debug=False,
                   num_devices=NCORES)

    # runtime inputs (image/unary-derived only)
    rgbsplit = nc.dram_tensor("rgbsplit", [9, N], bf16, kind="ExternalInput")
    biasB = nc.dram_tensor("biasB", [CHUNKS, 128], f32, kind="ExternalInput")
    psiS_u = nc.dram_tensor("psiS_u", [9, R], bf16, kind="ExternalInput")
    psiB_u = nc.dram_tensor("psiB_u", [18, R], bf16, kind="ExternalInput")
    u_rt = nc.dram_tensor("u_rt", [R, C], mybir.dt.float16, kind="ExternalInput")
    outT = nc.dram_tensor("outT", [C, R], f32, kind="ExternalOutput")

    # input-independent constants baked into the NEFF
    phiS_np, _, biasS_np = _spatial_consts()
    phiS_c = nc.inline_tensor(np.ascontiguousarray(phiS_np), name="phiS_c")
    biasS_c = nc.inline_tensor(np.ascontiguousarray(biasS_np), name="biasS_c")
    xy80_c = nc.inline_tensor(np.ascontiguousarray(_xy80_const()), name="xy80_c")

    n_stream = CHUNKS - SBUF_CHUNKS
    kmat = nc.dram_tensor("kmat", [n_stream, 128, R], bf16)
    qcc_in = nc.dram_tensor("qcc_in", [R, C], bf16)
    qcc_out = nc.dram_tensor("qcc_out", [N, C], bf16, addr_space="Shared")

    with tile.TileContext(nc) as tc:
        with tc.tile_pool(name="const", bufs=1) as const, \
             tc.tile_pool(name="kres", bufs=1) as kres_pool:

            ident = const.tile([128, 128], f32)
            make_identity(nc, ident)

            u16_sb = const.tile([128, RTILES, C], mybir.dt.float16)
            nc.sync.dma_start(
                out=u16_sb[:, 0:RT_FULL, :],
                in_=u_rt.ap()[0 : RT_FULL * 128].rearrange("(j p) c -> p j c", p=128),
            )
            nc.sync.dma_start(
                out=u16_sb[0:RT_LAST, RT_FULL, :],
                in_=u_rt.ap()[RT_FULL * 128 : R],
            )
            u_sb = const.tile([128, RTILES, C], f32)
            nc.vector.tensor_copy(out=u_sb, in_=u16_sb)

            # ---------------- K construction ----------------
            kres = []
            with tc.tile_pool(name="kbuild", bufs=3) as kbuild, \
                 tc.tile_pool(name="ksetup", bufs=1) as ksetup, \
                 tc.tile_pool(name="kpsum", bufs=2, space="PSUM") as kpsum:
                # psiS [21, R]: [a, b, c, a, b, c, a, b, c, L3]
                psiS_sb = ksetup.tile([PS_ROWS, R], bf16)
                for rep in range(3):
                    nc.sync.dma_start(out=psiS_sb[6 * rep : 6 * rep + 6, :],
                                      in_=psiS_u.ap()[0:6])
                nc.sync.dma_start(out=psiS_sb[18:21, :], in_=psiS_u.ap()[6:9])
                # psiB [48, R]: [a5, b5, c5] x3 + L3
                psiB_sb = ksetup.tile([PB_ROWS, R], bf16)
                for rep in range(3):
                    nc.sync.dma_start(out=psiB_sb[15 * rep : 15 * rep + 15, :],
                                      in_=psiB_u.ap()[0:15])
                nc.sync.dma_start(out=psiB_sb[45:48, :], in_=psiB_u.ap()[15:18])
                # phiB [48, N]: rows 15*lvl + 5*rep = [x80, y80, r, g, b](lvl)
                phiB_sb = ksetup.tile([PB_ROWS, N], bf16)
                for lvl in range(3):
                    for rep in range(3):
                        r0 = 15 * lvl + 5 * rep
                        nc.sync.dma_start(out=phiB_sb[r0 : r0 + 2, :],
                                          in_=xy80_c.ap()[2 * lvl : 2 * lvl + 2])
                        nc.sync.dma_start(out=phiB_sb[r0 + 2 : r0 + 5, :],
                                          in_=rgbsplit.ap()[3 * lvl : 3 * lvl + 3])
                nc.sync.dma_start(out=phiB_sb[45:48, :], in_=xy80_c.ap()[6:9])

                biasS_sb = ksetup.tile([128, CHUNKS], f32)
                nc.sync.dma_start(out=biasS_sb,
                                  in_=biasS_c.ap().rearrange("q p -> p q"))
                biasB_sb = ksetup.tile([128, CHUNKS], f32)
                nc.sync.dma_start(out=biasB_sb,
                                  in_=biasB.ap().rearrange("q p -> p q"))

                for q in range(CHUNKS):
                    phis_t = kbuild.tile([PS_ROWS, 128], bf16, tag="phis")
                    nc.sync.dma_start(out=phis_t,
                                      in_=phiS_c.ap()[:, q * 128 : (q + 1) * 128])

                    if q < SBUF_CHUNKS:
                        ktarget = kres_pool.tile([128, R], bf16, tag=f"k{q}")
                        kres.append(ktarget)
                    else:
                        ktarget = kbuild.tile([128, R], bf16, tag="kstage")

                    for (b, w) in BLOCKS:
                        ps = kpsum.tile([128, 512], f32, tag="ps")
                        nc.tensor.matmul(out=ps[:, 0:w], lhsT=phis_t,
                                         rhs=psiS_sb[:, b : b + w],
                                         start=True, stop=True)
                        pb = kpsum.tile([128, 512], f32, tag="pb")
                        nc.tensor.matmul(
                            out=pb[:, 0:w],
                            lhsT=phiB_sb[:, q * 128 : (q + 1) * 128],
                            rhs=psiB_sb[:, b : b + w],
                            start=True, stop=True)
                        es = kbuild.tile([128, 512], f32, tag="es")
                        nc.scalar.activation(out=es[:, 0:w], in_=ps[:, 0:w], func=EXP,
                                             bias=biasS_sb[:, q : q + 1], scale=1.0)
                        eb = kbuild.tile([128, 512], f32, tag="eb")
                        nc.scalar.activation(out=eb[:, 0:w], in_=pb[:, 0:w], func=EXP,
                                             bias=biasB_sb[:, q : q + 1], scale=1.0)
                        nc.vector.tensor_add(out=ktarget[:, b : b + w],
                                             in0=es[:, 0:w], in1=eb[:, 0:w])
                    if q >= SBUF_CHUNKS:
                        nc.sync.dma_start(out=kmat.ap()[q - SBUF_CHUNKS], in_=ktarget)

            # ---------------- Q0, iterations, output ----------------
            with tc.tile_pool(name="small", bufs=4) as small, \
                 tc.tile_pool(name="qbuf", bufs=2) as qbuf, \
                 tc.tile_pool(name="kstream", bufs=4) as kstream, \
                 tc.tile_pool(name="mtp", bufs=1) as mtp, \
                 tc.tile_pool(name="mm_psum", bufs=1, space="PSUM") as mm_psum, \
                 tc.tile_pool(name="tp_psum", bufs=2, space="PSUM") as tp_psum:

                qf_sb = const.tile([128, RTILES, C], f32)   # current Q shard f32
                g_sb = const.tile([128, RTILES, C], f32)    # U + 13*Q_prev(bf16)

                def softmax_tile(j, src_psum):
                    """src_psum None -> Q0 from -U; else PSUM tile [w, C] = M^T."""
                    w = 128 if j < RT_FULL else RT_LAST
                    ssum = small.tile([128, 1], f32, tag="ssum")
                    e = small.tile([128, C], f32, tag="e")
                    if src_psum is None:
                        mn = small.tile([128, 1], f32, tag="mn")
                        nc.vector.tensor_reduce(out=mn[0:w], in_=u_sb[0:w, j, :],
                                                axis=AX, op=OP.min)
                        nc.scalar.activation(out=e[0:w], in_=u_sb[0:w, j, :],
                                             func=EXP, bias=mn[0:w], scale=-1.0,
                                             accum_out=ssum[0:w])
                    else:
                        a = small.tile([128, C], f32, tag="a")
                        nc.vector.tensor_sub(out=a[0:w], in0=src_psum[0:w],
                                             in1=g_sb[0:w, j, :])
                        mn = small.tile([128, 1], f32, tag="mn")
                        nc.vector.tensor_reduce(out=mn[0:w], in_=a[0:w],
                                                axis=AX, op=OP.max, negate=True)
                        nc.scalar.activation(out=e[0:w], in_=a[0:w], func=EXP,
                                             bias=mn[0:w], scale=1.0,
                                             accum_out=ssum[0:w])
                    rcp = small.tile([128, 1], f32, tag="rcp")
                    nc.vector.reciprocal(out=rcp[0:w], in_=ssum[0:w])
                    nc.vector.tensor_scalar_mul(out=qf_sb[0:w, j, :], in0=e[0:w],
                                                scalar1=rcp[0:w])

                def post_softmax(last):
                    """bf16 copy of Q, update g, DMA out, AllGather, reload Qfull."""
                    qcc_sb = qbuf.tile([128, RTILES, C], bf16, tag="qcc")
                    nc.vector.tensor_copy(out=qcc_sb, in_=qf_sb)
                    if last:
                        return None
                    nc.vector.scalar_tensor_tensor(out=g_sb, in0=qcc_sb,
                                                   scalar=13.0, in1=u_sb,
                                                   op0=OP.mult, op1=OP.add)
                    nc.sync.dma_start(
                        out=qcc_in.ap()[0 : RT_FULL * 128].rearrange(
                            "(j p) c -> p j c", p=128),
                        in_=qcc_sb[:, 0:RT_FULL, :],
                    )
                    nc.sync.dma_start(
                        out=qcc_in.ap()[RT_FULL * 128 : R],
                        in_=qcc_sb[0:RT_LAST, RT_FULL, :],
                    )
                    nc.gpsimd.collective_compute(
                        "AllGather", OP.bypass,
                        replica_groups=[list(range(NCORES))],
                        ins=[qcc_in.ap().opt()],
                        outs=[qcc_out.ap().opt()],
                    )
                    qfull = qbuf.tile([128, CHUNKS, C], bf16, tag="qfull")
                    nc.sync.dma_start(
                        out=qfull,
                        in_=qcc_out.ap().rearrange("(q p) c -> p q c", p=128),
                    )
                    return qfull

                for j in range(RTILES):
                    softmax_tile(j, None)
                qfull = post_softmax(last=(iters == 0))

                for t in range(1, iters + 1):
                    mm = mm_psum.tile([C, R], f32, tag="mm")
                    for q in range(CHUNKS):
                        if q < SBUF_CHUNKS:
                            ktile = kres[q]
                        else:
                            ktile = kstream.tile([128, R], bf16, tag="kst")
                            nc.sync.dma_start(out=ktile,
                                              in_=kmat.ap()[q - SBUF_CHUNKS])
                        for (b, w) in BLOCKS:
                            nc.tensor.matmul(out=mm[:, b : b + w],
                                             lhsT=qfull[:, q, :],
                                             rhs=ktile[:, b : b + w],
                                             start=(q == 0),
                                             stop=(q == CHUNKS - 1))
                    mt = mtp.tile([C, R], f32, tag="mt")
                    nc.scalar.copy(out=mt, in_=mm)
                    for j in range(RTILES):
                        w = 128 if j < RT_FULL else RT_LAST
                        tp = tp_psum.tile([128, C], f32, tag="tp")
                        nc.tensor.transpose(out=tp[0:w],
                                            in_=mt[:, j * 128 : j * 128 + w],
                                            identity=ident[0:C, 0:C])
                        softmax_tile(j, tp)
                    qfull = post_softmax(last=(t == iters))

                outsb = mtp.tile([C, R], f32, tag="outsb")
                for j in range(RTILES):
                    w = 128 if j < RT_FULL else RT_LAST
                    tpo = tp_psum.tile([C, 128], f32, tag="tpo")
                    nc.tensor.transpose(out=tpo[:, 0:w], in_=qf_sb[0:w, j, :],
                                        identity=ident[0:w, 0:w])
                    nc.scalar.copy(out=outsb[:, j * 128 : j * 128 + w],
                                   in_=tpo[:, 0:w])
                nc.sync.dma_start(out=outT.ap(), in_=outsb)

    nc.finalize()
    return nc


class _Runner:
    """Caches the jitted SPMD callable for one compiled program."""

    def __init__(self, nc):
        import jax
        from concourse import bass2jax, mybir
        from jax.experimental.shard_map import shard_map
        from jax.sharding import Mesh, PartitionSpec

        bass2jax.install_neuronx_cc_hook()
        self.nc = nc

        in_names, out_names, out_avals = [], [], []
        part_name = nc.partition_id_tensor.name if nc.partition_id_tensor else None
        for alloc in nc.m.functions[0].allocations:
            if not isinstance(alloc, mybir.MemoryLocationSet):
                continue
            if alloc.kind not in ("ExternalInput", "ExternalOutput"):
                continue
            name = alloc.memorylocations[0].name
            if alloc.kind == "ExternalInput":
                if name != part_name:
                    in_names.append(name)
            else:
                out_names.append(name)
                out_avals.append(jax.core.ShapedArray(
                    tuple(alloc.tensor_shape), mybir.dt.np(alloc.dtype)))
        self.in_names = list(in_names)
        self.out_names = list(out_names)
        self.out_avals = out_avals
        n_params = len(in_names)
        n_outs = len(out_names)

        all_names = in_names + out_names
        if part_name is not None:
            all_names = all_names + [part_name]

        def _body(*args):
            operands = list(args)
            if part_name is not None:
                operands.append(bass2jax.partition_id_tensor())
            outs = bass2jax._bass_exec_p.bind(
                *operands,
                out_avals=tuple(out_avals),
                in_names=tuple(all_names),
                out_names=tuple(out_names),
                lowering_input_output_aliases=(),
                sim_require_finite=True,
                sim_require_nnan=True,
                nc=nc,
            )
            return tuple(outs)

        devices = jax.devices()[:NCORES]
        mesh = Mesh(np.asarray(devices), ("core",))
        in_specs = tuple(
            PartitionSpec() if name in _REPLICATED else PartitionSpec("core")
            for name in in_names
        ) + (PartitionSpec("core"),) * n_outs
        out_specs = (PartitionSpec("core"),) * n_outs
        donate = tuple(range(n_params, n_params + n_outs))
        self._fn = jax.jit(
            shard_map(_body, mesh=mesh, in_specs=in_specs, out_specs=out_specs,
                      check_rep=False),
            donate_argnums=donate,
            keep_unused=True,
        )

    def __call__(self, common, per_core):
        args = []
        for name in self.in_names:
            if name in _REPLICATED:
                args.append(common[name])
            else:
                args.append(np.concatenate([m[name] for m in per_core], axis=0))
        for aval in self.out_avals:
            args.append(np.zeros((NCORES * aval.shape[0], *aval.shape[1:]),
                                 aval.dtype))
        outs = self._fn(*args)
        return {
            name: np.asarray(outs[i]).reshape(NCORES, *self.out_avals[i].shape)
            for i, name in enumerate(self.out_names)
        }


def _get_runner(iters):
    if iters not in _runner_cache:
        _runner_cache[iters] = _Runner(_build_program(iters))
    return _runner_cache[iters]


def _host_inputs(unary, image):
    rgb = image.astype(np.float64).reshape(N, 3) * 255.0
    frgb = (rgb - 127.5) / THETA_BETA                     # [N, 3]
    rh, rm, rl = _split3(frgb.T)                          # [3, N] each
    rgbsplit = np.concatenate([rh, rm, rl], 0).astype(BF)  # [9, N]

    fxy = _pos_features() / THETA_ALPHA
    fb = np.concatenate([fxy, frgb], 1)                   # [N, 5]
    Lb = -0.5 * (fb * fb).sum(1)
    biasB = (Lb + math.log(W_BILATERAL)).astype(np.float32).reshape(CHUNKS, 128)

    # per-core psi uniques
    ab, bb, cb = _split3(fb.T)                            # [5, N]
    Lbh, Lbm, Lbl = _split3(Lb)
    psiB_u = np.concatenate([ab, bb, cb, Lbh[None], Lbm[None], Lbl[None]],
                            0).astype(BF)                 # [18, N]
    _, psiS_u_full, _ = _spatial_consts()                 # [9, N]

    U_T = np.ascontiguousarray(unary.reshape(C, N).T.astype(np.float32))

    common = {
        "rgbsplit": np.ascontiguousarray(rgbsplit),
        "biasB": np.ascontiguousarray(biasB),
    }
    per_core = []
    for core in range(NCORES):
        lo, hi = core * R, (core + 1) * R
        per_core.append({
            "psiS_u": np.ascontiguousarray(psiS_u_full[:, lo:hi]),
            "psiB_u": np.ascontiguousarray(psiB_u[:, lo:hi]),
            "u_rt": np.ascontiguousarray(U_T[lo:hi].astype(np.float16)),
        })
    return common, per_core


def kernel(unary, image, num_iterations):
    iters = int(num_iterations)
    unary = np.asarray(unary, dtype=np.float32)
    image = np.asarray(image, dtype=np.float32)

    runner = _get_runner(iters)
    common, per_core = _host_inputs(unary, image)
    outs = runner(common, per_core)["outT"]            # [NCORES, C, R]
    out = np.concatenate(list(outs), axis=1)           # [C, N]
    return np.ascontiguousarray(out.reshape(C, H_IMG, W_IMG).astype(np.float32))
